# revision 14
# baseline (speedup 1.0000x reference)
"""Multi-head attention (B=4, S=2048, D=1024, H=16) on 8 TRN2 NeuronCores.

Sharding: core c = (batch b = c//2, head-half g = c%2). Each core computes
8 heads of one batch: QKV projections (fp16), logits (fp16, K=64), softmax
with multiplicative inverse-mask (exact zeros for masked entries, matching
the reference's additive float32.min mask), attention weights written to HBM
in fp16 (upcast to f32 during the host-side gather), context via
PE-transposed fp16 P, and a partial output projection (fp16) whose
core-pair partials are summed on the host (Wo row split).

Measured on trn2: HW exec ~710 us across 8 cores; max rel err ~1e-3
(out 8.2e-4, attn 9.8e-4) vs the fp32 jax reference.
"""

import sys

sys.path.insert(0, "/opt/trn_rl_repo")

import numpy as np

import concourse.bass as bass  # noqa: F401  (engine types referenced via nc)
from concourse import bacc
import concourse.tile as tile
from concourse import mybir
from concourse.bass_utils import run_bass_kernel_spmd
from concourse.masks import make_identity

F32 = mybir.dt.float32
F16 = mybir.dt.float16

B, S, D, H = 4, 2048, 1024, 16
DH = D // H          # 64
HL = H // 2          # 8 heads per core
DL = HL * DH         # 512 local dims per core
N_CORES = 8

_compiled_nc = None


def _build_nc():
    """Build + compile the per-core Bass kernel (SPMD: same program, 8 cores)."""
    nc = bacc.Bacc(None, target_bir_lowering=False)

    xqt = nc.declare_dram_parameter("xqt", [D, S], F16, isOutput=False)
    xkt = nc.declare_dram_parameter("xkt", [D, S], F16, isOutput=False)
    xvt = nc.declare_dram_parameter("xvt", [D, S], F16, isOutput=False)
    wq = nc.declare_dram_parameter("wq", [D, DL], F16, isOutput=False)
    wk = nc.declare_dram_parameter("wk", [D, DL], F16, isOutput=False)
    wv = nc.declare_dram_parameter("wv", [D, DL], F16, isOutput=False)
    bq = nc.declare_dram_parameter("bq", [DL], F32, isOutput=False)
    bk = nc.declare_dram_parameter("bk", [DL], F32, isOutput=False)
    bv = nc.declare_dram_parameter("bv", [1, DL], F32, isOutput=False)
    wo = nc.declare_dram_parameter("wo", [DL, D], F16, isOutput=False)
    maskinv = nc.declare_dram_parameter("maskinv", [S, S], F16, isOutput=False)

    attn = nc.declare_dram_parameter("attn", [HL, S, S], F16, isOutput=True)
    outp = nc.declare_dram_parameter("outp", [S, D], F32, isOutput=True)

    QB = S // 128      # 16 query blocks per head
    KB = S // 512      # 4 key blocks
    TB = S // 512      # 4 token blocks (projection)
    KC = S // 128      # 16 key chunks (ctx contraction)
    CH = D // 128      # 8 contraction chunks (projection)
    PAIRS = DL // 128  # 4 head pairs

    with tile.TileContext(nc) as tc:
        with (
            tc.tile_pool(name="persist", bufs=1) as persist,
            tc.tile_pool(name="small", bufs=1) as small,
        ):
            # --- persistent SBUF tensors ---
            qT16 = persist.tile([128, PAIRS, S], F16, tag="qT16")    # 2 MB
            kT16 = persist.tile([128, PAIRS, S], F16, tag="kT16")    # 2 MB
            v16 = persist.tile([128, KC, DL], F16, tag="v16")        # 2 MB
            ctxT16 = persist.tile([128, PAIRS, S], F16, tag="ctxT16")  # 2 MB
            wo16 = persist.tile([128, PAIRS, D], F16, tag="wo16")    # 1 MB
            mk = persist.tile([128, QB, S], F16, tag="mk")           # 8 MB

            ident = small.tile([128, 128], F16, tag="ident")
            ones1 = small.tile([1, 128], F16, tag="ones1")
            bq_sb = small.tile([128, PAIRS], F32, tag="bq_sb")
            bk_sb = small.tile([128, PAIRS], F32, tag="bk_sb")
            bv_f32 = small.tile([1, DL], F32, tag="bv_f32")
            bv16 = small.tile([1, DL], F16, tag="bv16")

            make_identity(nc, ident)
            nc.vector.memset(ones1, 1.0)
            nc.sync.dma_start(bq_sb, bq[:].rearrange("(d p) -> p d", p=128))
            nc.sync.dma_start(bk_sb, bk[:].rearrange("(d p) -> p d", p=128))
            nc.sync.dma_start(bv_f32, bv[:])
            nc.vector.tensor_copy(bv16, bv_f32)
            # big mask load (SWDGE: single-sem, consumed by DVE)
            nc.gpsimd.dma_start(mk, maskinv[:].rearrange("(q p) k -> p q k", p=128))
            nc.gpsimd.dma_start(wo16, wo[:].rearrange("(r p) d -> p r d", p=128))

            # ---------------- Phase A: projections ----------------
            with (
                tc.tile_pool(name="wpool", bufs=2) as wpool,
                tc.tile_pool(name="xpool", bufs=3) as xpool,
                tc.tile_pool(name="pa", bufs=2, space="PSUM") as pa,
            ):
                for which, wdram, xdram in (
                    ("q", wq, xqt),
                    ("k", wk, xkt),
                    ("v", wv, xvt),
                ):
                    w_sb = wpool.tile([128, CH, DL], F16, tag="w")
                    nc.sync.dma_start(
                        w_sb, wdram[:].rearrange("(c p) n -> p c n", p=128)
                    )
                    for t in range(TB):
                        xt = xpool.tile([128, CH, 512], F16, tag="xt")
                        nc.sync.dma_start(
                            xt,
                            xdram[:, t * 512:(t + 1) * 512]
                            .rearrange("(c p) s -> p c s", p=128),
                        )
                        if which in ("q", "k"):
                            dst = qT16 if which == "q" else kT16
                            bias = bq_sb if which == "q" else bk_sb
                            for d in range(PAIRS):
                                ps = pa.tile([128, 512], F32, tag="pa")
                                for c in range(CH):
                                    nc.tensor.matmul(
                                        ps,
                                        w_sb[:, c, d * 128:(d + 1) * 128],
                                        xt[:, c, :],
                                        start=(c == 0),
                                        stop=(c == CH - 1),
                                    )
                                nc.vector.tensor_scalar(
                                    out=dst[:, d, t * 512:(t + 1) * 512],
                                    in0=ps,
                                    scalar1=bias[:, d:d + 1],
                                    scalar2=None,
                                    op0=mybir.AluOpType.add,
                                )
                        else:
                            for tt in range(4):  # token tiles inside block
                                ps = pa.tile([128, 512], F32, tag="pa")
                                for c in range(CH):
                                    nc.tensor.matmul(
                                        ps,
                                        xt[:, c, tt * 128:(tt + 1) * 128],
                                        w_sb[:, c, :],
                                        start=(c == 0),
                                        stop=False,
                                    )
                                nc.tensor.matmul(
                                    ps, ones1, bv16, start=False, stop=True
                                )
                                nc.scalar.copy(v16[:, t * 4 + tt, :], ps)

            # ---------------- Phase B: attention ----------------
            with (
                tc.tile_pool(name="pb_l", bufs=2, space="PSUM") as pb_l,
                tc.tile_pool(name="pb_t", bufs=2, space="PSUM") as pb_t,
                tc.tile_pool(name="pb_c", bufs=1, space="PSUM") as pb_c,
                tc.tile_pool(name="praw_p", bufs=3) as praw_p,
                tc.tile_pool(name="p16_p", bufs=4) as p16_p,
                tc.tile_pool(name="pT_p", bufs=2) as pT_p,
                tc.tile_pool(name="rs_p", bufs=8) as rs_p,
                tc.tile_pool(name="osb_p", bufs=3) as osb_p,
            ):
                for qg in range(QB // 4):       # groups of 4 q-stripes
                    for h in range(HL):
                        pr = h // 2
                        p0 = (h % 2) * 64
                        pT16 = pT_p.tile([128, KC, 512], F16, tag="pT16")
                        for qi in range(4):
                            qb = qg * 4 + qi
                            praw = praw_p.tile([128, KB, 512], F16, tag="praw")
                            for half in range(2):
                                psl = pb_l.tile([128, 2, 512], F32, tag="psl")
                                for kk in range(2):
                                    kb = half * 2 + kk
                                    nc.tensor.matmul(
                                        psl[:, kk, :],
                                        qT16[p0:p0 + 64, pr, qb * 128:(qb + 1) * 128],
                                        kT16[p0:p0 + 64, pr, kb * 512:(kb + 1) * 512],
                                        start=True,
                                        stop=True,
                                    )
                                nc.scalar.activation(
                                    praw[:, half * 2:(half + 1) * 2, :],
                                    psl,
                                    mybir.ActivationFunctionType.Exp,
                                )
                            p16 = p16_p.tile([128, S], F16, tag="p16")
                            rs = rs_p.tile([128, 1], F32, tag="rs")
                            nc.vector.scalar_tensor_tensor(
                                out=p16.rearrange("p (a b) -> p a b", a=KB),
                                in0=praw,
                                scalar=1.0,
                                in1=mk[:, qb, :].rearrange("p (a b) -> p a b", a=KB),
                                op0=mybir.AluOpType.mult,
                                op1=mybir.AluOpType.mult,
                                accum_out=rs,
                            )
                            rcp = rs_p.tile([128, 1], F32, tag="rcp")
                            nc.vector.reciprocal(rcp, rs)
                            nc.vector.tensor_scalar_mul(p16, p16, rcp)
                            nc.sync.dma_start(
                                attn[h, qb * 128:(qb + 1) * 128, :], p16
                            )
                            for kb in range(KB):
                                pst = pb_t.tile([128, 4, 128], F16, tag="pst")
                                for j in range(4):
                                    nc.tensor.transpose(
                                        pst[:, j, :],
                                        p16[:, (kb * 4 + j) * 128:(kb * 4 + j + 1) * 128],
                                        ident,
                                    )
                                nc.any.tensor_copy(
                                    pT16[:, kb * 4:(kb + 1) * 4, qi * 128:(qi + 1) * 128],
                                    pst,
                                )
                        psc = pb_c.tile([64, 512], F32, tag="psc")
                        for kc in range(KC):
                            nc.tensor.matmul(
                                psc,
                                v16[:, kc, h * 64:(h + 1) * 64],
                                pT16[:, kc, :],
                                start=(kc == 0),
                                stop=(kc == KC - 1),
                            )
                        nc.scalar.copy(
                            ctxT16[p0:p0 + 64, pr, qg * 512:(qg + 1) * 512], psc
                        )
                    # output projection for this q-group (all heads done)
                    for qt in range(4):
                        q0 = qg * 512 + qt * 128
                        for n in range(2):
                            pso = pb_c.tile([128, 512], F32, tag="pso")
                            for p4 in range(PAIRS):
                                nc.tensor.matmul(
                                    pso,
                                    ctxT16[:, p4, q0:q0 + 128],
                                    wo16[:, p4, n * 512:(n + 1) * 512],
                                    start=(p4 == 0),
                                    stop=(p4 == PAIRS - 1),
                                )
                            osb = osb_p.tile([128, 512], F32, tag="osb")
                            nc.any.tensor_copy(osb, pso)
                            nc.sync.dma_start(
                                outp[q0:q0 + 128, n * 512:(n + 1) * 512], osb
                            )

    nc.compile()
    return nc


def _get_nc():
    global _compiled_nc
    if _compiled_nc is None:
        _compiled_nc = _build_nc()
    return _compiled_nc


def _prep_in_maps(queries, keys, values, mask, Wq, bq, Wk, bk, Wv, bv, Wo, bo):
    """Host-side sharding / layout prep. Core c = (batch c//2, head-half c%2)."""
    scale = 1.0 / np.sqrt(np.float32(DH))
    maskinv_all = (1.0 - mask[:, 0]).astype(np.float16)  # (B,S,S) 1=keep
    xq = [np.ascontiguousarray(queries[b].T).astype(np.float16) for b in range(B)]
    xk = [np.ascontiguousarray(keys[b].T).astype(np.float16) for b in range(B)]
    xv = [np.ascontiguousarray(values[b].T).astype(np.float16) for b in range(B)]
    in_maps = []
    for c in range(N_CORES):
        b, g = c // 2, c % 2
        cols = slice(g * DL, (g + 1) * DL)
        in_maps.append({
            "xqt": xq[b],
            "xkt": xk[b],
            "xvt": xv[b],
            "wq": (Wq[:, cols] * scale).astype(np.float16),
            "wk": Wk[:, cols].astype(np.float16),
            "wv": Wv[:, cols].astype(np.float16),
            "bq": np.ascontiguousarray(bq[cols] * scale),
            "bk": np.ascontiguousarray(bk[cols]),
            "bv": np.ascontiguousarray(bv[cols])[None, :],
            "wo": np.ascontiguousarray(Wo[g * DL:(g + 1) * DL, :]).astype(np.float16),
            "maskinv": maskinv_all[b],
        })
    return in_maps


def _run(inputs, trace=False, trace_kwargs=None):
    inputs = {k: np.asarray(v, dtype=np.float32) for k, v in inputs.items()}
    nc = _get_nc()
    in_maps = _prep_in_maps(**inputs)
    kw = {}
    if trace:
        kw = dict(trace=True, trace_kwargs=trace_kwargs or {})
    res = run_bass_kernel_spmd(nc, in_maps, list(range(N_CORES)), **kw)

    attn_w = np.empty((B, H, S, S), np.float32)
    out = np.empty((B, S, D), np.float32)
    bo = inputs["bo"]
    for c in range(N_CORES):
        b, g = c // 2, c % 2
        attn_w[b, g * HL:(g + 1) * HL] = res.results[c]["attn"].astype(np.float32)
    for b in range(B):
        out[b] = res.results[2 * b]["outp"] + res.results[2 * b + 1]["outp"] + bo
    return (out, attn_w), res


def kernel(**inputs):
    (out, attn_w), _ = _run(inputs, trace=False)
    return out, attn_w


# revision 15
# speedup vs baseline: 1.0344x; 1.0344x over previous
"""Multi-head attention (B=4, S=2048, D=1024, H=16) on 8 TRN2 NeuronCores.

Sharding: core c = (batch b = c//2, head-half g = c%2). Each core computes
8 heads of one batch: QKV projections (fp16), logits (fp16, K=64), softmax
with multiplicative inverse-mask (exact zeros for masked entries, matching
the reference's additive float32.min mask), attention weights written to HBM
in fp16 (upcast to f32 during the host-side gather), context via
PE-transposed fp16 P, and a partial output projection (fp16) whose
core-pair partials are summed on the host (Wo row split).

Measured on trn2: HW exec ~710 us across 8 cores; max rel err ~1e-3
(out 8.2e-4, attn 9.8e-4) vs the fp32 jax reference.
"""

import sys

sys.path.insert(0, "/opt/trn_rl_repo")

import numpy as np

import concourse.bass as bass  # noqa: F401  (engine types referenced via nc)
from concourse import bacc
import concourse.tile as tile
from concourse import mybir
from concourse.bass_utils import run_bass_kernel_spmd
from concourse.masks import make_identity

F32 = mybir.dt.float32
F16 = mybir.dt.float16

B, S, D, H = 4, 2048, 1024, 16
DH = D // H          # 64
HL = H // 2          # 8 heads per core
DL = HL * DH         # 512 local dims per core
N_CORES = 8

_compiled_nc = None


def _build_nc():
    """Build + compile the per-core Bass kernel (SPMD: same program, 8 cores)."""
    nc = bacc.Bacc(None, target_bir_lowering=False)

    xqt = nc.declare_dram_parameter("xqt", [D, S], F16, isOutput=False)
    xkt = nc.declare_dram_parameter("xkt", [D, S], F16, isOutput=False)
    xvt = nc.declare_dram_parameter("xvt", [D, S], F16, isOutput=False)
    wq = nc.declare_dram_parameter("wq", [D, DL], F16, isOutput=False)
    wk = nc.declare_dram_parameter("wk", [D, DL], F16, isOutput=False)
    wv = nc.declare_dram_parameter("wv", [D, DL], F16, isOutput=False)
    bq = nc.declare_dram_parameter("bq", [DL], F32, isOutput=False)
    bk = nc.declare_dram_parameter("bk", [DL], F32, isOutput=False)
    bv = nc.declare_dram_parameter("bv", [1, DL], F32, isOutput=False)
    wo = nc.declare_dram_parameter("wo", [DL, D], F16, isOutput=False)
    maskinv = nc.declare_dram_parameter("maskinv", [S, S], F16, isOutput=False)

    attn = nc.declare_dram_parameter("attn", [HL, S, S], F16, isOutput=True)
    outp = nc.declare_dram_parameter("outp", [S, D], F32, isOutput=True)

    QB = S // 128      # 16 query blocks per head
    KB = S // 512      # 4 key blocks
    TB = S // 512      # 4 token blocks (projection)
    KC = S // 128      # 16 key chunks (ctx contraction)
    CH = D // 128      # 8 contraction chunks (projection)
    PAIRS = DL // 128  # 4 head pairs

    with tile.TileContext(nc) as tc:
        with (
            tc.tile_pool(name="persist", bufs=1) as persist,
            tc.tile_pool(name="small", bufs=1) as small,
        ):
            # --- persistent SBUF tensors ---
            qT16 = persist.tile([128, PAIRS, S], F16, tag="qT16")    # 2 MB
            kT16 = persist.tile([128, PAIRS, S], F16, tag="kT16")    # 2 MB
            v16 = persist.tile([128, KC, DL], F16, tag="v16")        # 2 MB
            ctxT16 = persist.tile([128, PAIRS, S], F16, tag="ctxT16")  # 2 MB
            wo16 = persist.tile([128, PAIRS, D], F16, tag="wo16")    # 1 MB
            mk = persist.tile([128, QB, S], F16, tag="mk")           # 8 MB

            ident = small.tile([128, 128], F16, tag="ident")
            ones1 = small.tile([1, 128], F16, tag="ones1")
            bq_sb = small.tile([128, PAIRS], F32, tag="bq_sb")
            bk_sb = small.tile([128, PAIRS], F32, tag="bk_sb")
            bv_f32 = small.tile([1, DL], F32, tag="bv_f32")
            bv16 = small.tile([1, DL], F16, tag="bv16")

            make_identity(nc, ident)
            nc.vector.memset(ones1, 1.0)
            nc.sync.dma_start(bq_sb, bq[:].rearrange("(d p) -> p d", p=128))
            nc.sync.dma_start(bk_sb, bk[:].rearrange("(d p) -> p d", p=128))
            nc.sync.dma_start(bv_f32, bv[:])
            nc.vector.tensor_copy(bv16, bv_f32)
            # big mask load (SWDGE: single-sem, consumed by DVE)
            nc.gpsimd.dma_start(mk, maskinv[:].rearrange("(q p) k -> p q k", p=128))
            nc.gpsimd.dma_start(wo16, wo[:].rearrange("(r p) d -> p r d", p=128))

            # ---------------- Phase A: projections ----------------
            with (
                tc.tile_pool(name="wpool", bufs=2) as wpool,
                tc.tile_pool(name="xpool", bufs=3) as xpool,
                tc.tile_pool(name="pa", bufs=2, space="PSUM") as pa,
            ):
                for which, wdram, xdram in (
                    ("q", wq, xqt),
                    ("k", wk, xkt),
                    ("v", wv, xvt),
                ):
                    w_sb = wpool.tile([128, CH, DL], F16, tag="w")
                    nc.sync.dma_start(
                        w_sb, wdram[:].rearrange("(c p) n -> p c n", p=128)
                    )
                    for t in range(TB):
                        xt = xpool.tile([128, CH, 512], F16, tag="xt")
                        nc.sync.dma_start(
                            xt,
                            xdram[:, t * 512:(t + 1) * 512]
                            .rearrange("(c p) s -> p c s", p=128),
                        )
                        if which in ("q", "k"):
                            dst = qT16 if which == "q" else kT16
                            bias = bq_sb if which == "q" else bk_sb
                            for d in range(PAIRS):
                                ps = pa.tile([128, 512], F32, tag="pa")
                                for c in range(CH):
                                    nc.tensor.matmul(
                                        ps,
                                        w_sb[:, c, d * 128:(d + 1) * 128],
                                        xt[:, c, :],
                                        start=(c == 0),
                                        stop=(c == CH - 1),
                                    )
                                nc.vector.tensor_scalar(
                                    out=dst[:, d, t * 512:(t + 1) * 512],
                                    in0=ps,
                                    scalar1=bias[:, d:d + 1],
                                    scalar2=None,
                                    op0=mybir.AluOpType.add,
                                )
                        else:
                            for tt in range(4):  # token tiles inside block
                                ps = pa.tile([128, 512], F32, tag="pa")
                                for c in range(CH):
                                    nc.tensor.matmul(
                                        ps,
                                        xt[:, c, tt * 128:(tt + 1) * 128],
                                        w_sb[:, c, :],
                                        start=(c == 0),
                                        stop=False,
                                    )
                                nc.tensor.matmul(
                                    ps, ones1, bv16, start=False, stop=True
                                )
                                nc.scalar.copy(v16[:, t * 4 + tt, :], ps)

            # ---------------- Phase B: attention ----------------
            with (
                tc.tile_pool(name="pb_l", bufs=2, space="PSUM") as pb_l,
                tc.tile_pool(name="pb_t", bufs=3, space="PSUM") as pb_t,
                tc.tile_pool(name="pb_c", bufs=1, space="PSUM") as pb_c,
                tc.tile_pool(name="praw_p", bufs=4) as praw_p,
                tc.tile_pool(name="p16_p", bufs=4) as p16_p,
                tc.tile_pool(name="pT_p", bufs=2) as pT_p,
                tc.tile_pool(name="rs_p", bufs=8) as rs_p,
            ):
                for h in range(HL):
                    pr = h // 2
                    p0 = (h % 2) * 64
                    for qg in range(QB // 4):       # groups of 4 q-stripes
                        pT16 = pT_p.tile([128, KC, 512], F16, tag="pT16")
                        for qi in range(4):
                            qb = qg * 4 + qi
                            praw = praw_p.tile([128, KB, 512], F16, tag="praw")
                            for half in range(2):
                                psl = pb_l.tile([128, 2, 512], F32, tag="psl")
                                for kk in range(2):
                                    kb = half * 2 + kk
                                    nc.tensor.matmul(
                                        psl[:, kk, :],
                                        qT16[p0:p0 + 64, pr, qb * 128:(qb + 1) * 128],
                                        kT16[p0:p0 + 64, pr, kb * 512:(kb + 1) * 512],
                                        start=True,
                                        stop=True,
                                    )
                                nc.scalar.activation(
                                    praw[:, half * 2:(half + 1) * 2, :],
                                    psl,
                                    mybir.ActivationFunctionType.Exp,
                                )
                            p16 = p16_p.tile([128, S], F16, tag="p16")
                            rs = rs_p.tile([128, 1], F32, tag="rs")
                            nc.vector.scalar_tensor_tensor(
                                out=p16.rearrange("p (a b) -> p a b", a=KB),
                                in0=praw,
                                scalar=1.0,
                                in1=mk[:, qb, :].rearrange("p (a b) -> p a b", a=KB),
                                op0=mybir.AluOpType.mult,
                                op1=mybir.AluOpType.mult,
                                accum_out=rs,
                            )
                            rcp = rs_p.tile([128, 1], F32, tag="rcp")
                            nc.vector.reciprocal(rcp, rs)
                            nc.vector.tensor_scalar_mul(p16, p16, rcp)
                            nc.sync.dma_start(
                                attn[h, qb * 128:(qb + 1) * 128, :], p16
                            )
                            for kb in range(KB):
                                pst = pb_t.tile([128, 4, 128], F16, tag="pst")
                                for j in range(4):
                                    nc.tensor.transpose(
                                        pst[:, j, :],
                                        p16[:, (kb * 4 + j) * 128:(kb * 4 + j + 1) * 128],
                                        ident,
                                    )
                                nc.any.tensor_copy(
                                    pT16[:, kb * 4:(kb + 1) * 4, qi * 128:(qi + 1) * 128],
                                    pst,
                                )
                        psc = pb_c.tile([64, 512], F32, tag="psc")
                        for kc in range(KC):
                            nc.tensor.matmul(
                                psc,
                                v16[:, kc, h * 64:(h + 1) * 64],
                                pT16[:, kc, :],
                                start=(kc == 0),
                                stop=(kc == KC - 1),
                            )
                        nc.scalar.copy(
                            ctxT16[p0:p0 + 64, pr, qg * 512:(qg + 1) * 512], psc
                        )

            # ---------------- Phase C: output projection ----------------
            with (
                tc.tile_pool(name="pc", bufs=4, space="PSUM") as pc,
                tc.tile_pool(name="osb_p", bufs=3) as osb_p,
            ):
                for qt in range(QB):
                    for n in range(2):
                        ps = pc.tile([128, 512], F32, tag="pc")
                        for p4 in range(PAIRS):
                            nc.tensor.matmul(
                                ps,
                                ctxT16[:, p4, qt * 128:(qt + 1) * 128],
                                wo16[:, p4, n * 512:(n + 1) * 512],
                                start=(p4 == 0),
                                stop=(p4 == PAIRS - 1),
                            )
                        osb = osb_p.tile([128, 512], F32, tag="osb")
                        nc.any.tensor_copy(osb, ps)
                        nc.sync.dma_start(
                            outp[qt * 128:(qt + 1) * 128, n * 512:(n + 1) * 512],
                            osb,
                        )

    nc.compile()
    return nc


def _get_nc():
    global _compiled_nc
    if _compiled_nc is None:
        _compiled_nc = _build_nc()
    return _compiled_nc


def _prep_in_maps(queries, keys, values, mask, Wq, bq, Wk, bk, Wv, bv, Wo, bo):
    """Host-side sharding / layout prep. Core c = (batch c//2, head-half c%2)."""
    scale = 1.0 / np.sqrt(np.float32(DH))
    maskinv_all = (1.0 - mask[:, 0]).astype(np.float16)  # (B,S,S) 1=keep
    xq = [np.ascontiguousarray(queries[b].T).astype(np.float16) for b in range(B)]
    xk = [np.ascontiguousarray(keys[b].T).astype(np.float16) for b in range(B)]
    xv = [np.ascontiguousarray(values[b].T).astype(np.float16) for b in range(B)]
    in_maps = []
    for c in range(N_CORES):
        b, g = c // 2, c % 2
        cols = slice(g * DL, (g + 1) * DL)
        in_maps.append({
            "xqt": xq[b],
            "xkt": xk[b],
            "xvt": xv[b],
            "wq": (Wq[:, cols] * scale).astype(np.float16),
            "wk": Wk[:, cols].astype(np.float16),
            "wv": Wv[:, cols].astype(np.float16),
            "bq": np.ascontiguousarray(bq[cols] * scale),
            "bk": np.ascontiguousarray(bk[cols]),
            "bv": np.ascontiguousarray(bv[cols])[None, :],
            "wo": np.ascontiguousarray(Wo[g * DL:(g + 1) * DL, :]).astype(np.float16),
            "maskinv": maskinv_all[b],
        })
    return in_maps


def _run(inputs, trace=False, trace_kwargs=None):
    inputs = {k: np.asarray(v, dtype=np.float32) for k, v in inputs.items()}
    nc = _get_nc()
    in_maps = _prep_in_maps(**inputs)
    kw = {}
    if trace:
        kw = dict(trace=True, trace_kwargs=trace_kwargs or {})
    res = run_bass_kernel_spmd(nc, in_maps, list(range(N_CORES)), **kw)

    attn_w = np.empty((B, H, S, S), np.float32)
    out = np.empty((B, S, D), np.float32)
    bo = inputs["bo"]
    for c in range(N_CORES):
        b, g = c // 2, c % 2
        attn_w[b, g * HL:(g + 1) * HL] = res.results[c]["attn"].astype(np.float32)
    for b in range(B):
        out[b] = res.results[2 * b]["outp"] + res.results[2 * b + 1]["outp"] + bo
    return (out, attn_w), res


def kernel(**inputs):
    (out, attn_w), _ = _run(inputs, trace=False)
    return out, attn_w


# revision 16
# speedup vs baseline: 1.0440x; 1.0093x over previous
"""Multi-head attention (B=4, S=2048, D=1024, H=16) on 8 TRN2 NeuronCores.

Sharding: core c = (batch b = c//2, head-half g = c%2). Each core computes
8 heads of one batch: QKV projections (fp16), logits (fp16, K=64), softmax
with multiplicative inverse-mask (exact zeros for masked entries, matching
the reference's additive float32.min mask), attention weights written to HBM
in fp16 (upcast to f32 during the host-side gather), context via
PE-transposed fp16 P, and a partial output projection (fp16) whose
core-pair partials are summed on the host (Wo row split).

Measured on trn2: HW exec ~710 us across 8 cores; max rel err ~1e-3
(out 8.2e-4, attn 9.8e-4) vs the fp32 jax reference.
"""

import sys

sys.path.insert(0, "/opt/trn_rl_repo")

import numpy as np

import concourse.bass as bass  # noqa: F401  (engine types referenced via nc)
from concourse import bacc
import concourse.tile as tile
from concourse import mybir
from concourse.bass_utils import run_bass_kernel_spmd
from concourse.masks import make_identity

F32 = mybir.dt.float32
F16 = mybir.dt.float16

B, S, D, H = 4, 2048, 1024, 16
DH = D // H          # 64
HL = H // 2          # 8 heads per core
DL = HL * DH         # 512 local dims per core
N_CORES = 8

_compiled_nc = None


def _build_nc():
    """Build + compile the per-core Bass kernel (SPMD: same program, 8 cores)."""
    nc = bacc.Bacc(None, target_bir_lowering=False)

    xqt = nc.declare_dram_parameter("xqt", [D, S], F16, isOutput=False)
    xkt = nc.declare_dram_parameter("xkt", [D, S], F16, isOutput=False)
    xvt = nc.declare_dram_parameter("xvt", [D, S], F16, isOutput=False)
    wq = nc.declare_dram_parameter("wq", [D, DL], F16, isOutput=False)
    wk = nc.declare_dram_parameter("wk", [D, DL], F16, isOutput=False)
    wv = nc.declare_dram_parameter("wv", [D, DL], F16, isOutput=False)
    bq = nc.declare_dram_parameter("bq", [DL], F32, isOutput=False)
    bk = nc.declare_dram_parameter("bk", [DL], F32, isOutput=False)
    bv = nc.declare_dram_parameter("bv", [1, DL], F32, isOutput=False)
    wo = nc.declare_dram_parameter("wo", [DL, D], F16, isOutput=False)
    maskinv = nc.declare_dram_parameter("maskinv", [S, S], F16, isOutput=False)

    attn = nc.declare_dram_parameter("attn", [HL, S, S], F16, isOutput=True)
    outp = nc.declare_dram_parameter("outp", [S, D], F32, isOutput=True)

    QB = S // 128      # 16 query blocks per head
    KB = S // 512      # 4 key blocks
    TB = S // 512      # 4 token blocks (projection)
    KC = S // 128      # 16 key chunks (ctx contraction)
    CH = D // 128      # 8 contraction chunks (projection)
    PAIRS = DL // 128  # 4 head pairs

    with tile.TileContext(nc) as tc:
        with (
            tc.tile_pool(name="persist", bufs=1) as persist,
            tc.tile_pool(name="small", bufs=1) as small,
        ):
            # --- persistent SBUF tensors ---
            qT16 = persist.tile([128, PAIRS, S], F16, tag="qT16")    # 2 MB
            kT16 = persist.tile([128, PAIRS, S], F16, tag="kT16")    # 2 MB
            v16 = persist.tile([128, KC, DL], F16, tag="v16")        # 2 MB
            ctxT16 = persist.tile([128, PAIRS, S], F16, tag="ctxT16")  # 2 MB
            wo16 = persist.tile([128, PAIRS, D], F16, tag="wo16")    # 1 MB
            mk = persist.tile([128, QB, S], F16, tag="mk")           # 8 MB

            ident = small.tile([128, 128], F16, tag="ident")
            ones1 = small.tile([1, 128], F16, tag="ones1")
            bq_sb = small.tile([128, PAIRS], F32, tag="bq_sb")
            bk_sb = small.tile([128, PAIRS], F32, tag="bk_sb")
            bv_f32 = small.tile([1, DL], F32, tag="bv_f32")
            bv16 = small.tile([1, DL], F16, tag="bv16")

            make_identity(nc, ident)
            nc.vector.memset(ones1, 1.0)
            nc.sync.dma_start(bq_sb, bq[:].rearrange("(d p) -> p d", p=128))
            nc.sync.dma_start(bk_sb, bk[:].rearrange("(d p) -> p d", p=128))
            nc.sync.dma_start(bv_f32, bv[:])
            nc.vector.tensor_copy(bv16, bv_f32)

            # ---------------- Phase A: projections ----------------
            with (
                tc.tile_pool(name="wpool", bufs=2) as wpool,
                tc.tile_pool(name="xpool", bufs=3) as xpool,
                tc.tile_pool(name="pa", bufs=2, space="PSUM") as pa,
            ):
                for which, wdram, xdram in (
                    ("q", wq, xqt),
                    ("k", wk, xkt),
                    ("v", wv, xvt),
                ):
                    w_sb = wpool.tile([128, CH, DL], F16, tag="w")
                    nc.sync.dma_start(
                        w_sb, wdram[:].rearrange("(c p) n -> p c n", p=128)
                    )
                    for t in range(TB):
                        xt = xpool.tile([128, CH, 512], F16, tag="xt")
                        nc.sync.dma_start(
                            xt,
                            xdram[:, t * 512:(t + 1) * 512]
                            .rearrange("(c p) s -> p c s", p=128),
                        )
                        if which in ("q", "k"):
                            dst = qT16 if which == "q" else kT16
                            bias = bq_sb if which == "q" else bk_sb
                            for d in range(PAIRS):
                                ps = pa.tile([128, 512], F32, tag="pa")
                                for c in range(CH):
                                    nc.tensor.matmul(
                                        ps,
                                        w_sb[:, c, d * 128:(d + 1) * 128],
                                        xt[:, c, :],
                                        start=(c == 0),
                                        stop=(c == CH - 1),
                                    )
                                nc.vector.tensor_scalar(
                                    out=dst[:, d, t * 512:(t + 1) * 512],
                                    in0=ps,
                                    scalar1=bias[:, d:d + 1],
                                    scalar2=None,
                                    op0=mybir.AluOpType.add,
                                )
                        else:
                            for tt in range(4):  # token tiles inside block
                                ps = pa.tile([128, 512], F32, tag="pa")
                                for c in range(CH):
                                    nc.tensor.matmul(
                                        ps,
                                        xt[:, c, tt * 128:(tt + 1) * 128],
                                        w_sb[:, c, :],
                                        start=(c == 0),
                                        stop=False,
                                    )
                                nc.tensor.matmul(
                                    ps, ones1, bv16, start=False, stop=True
                                )
                                nc.scalar.copy(v16[:, t * 4 + tt, :], ps)

            # big mask + wo load: emitted after phase-A input loads so the
            # projection inputs win the DMA queues at kernel start (SWDGE,
            # single-sem, consumed by DVE / PE well after they complete)
            nc.gpsimd.dma_start(mk, maskinv[:].rearrange("(q p) k -> p q k", p=128))
            nc.gpsimd.dma_start(wo16, wo[:].rearrange("(r p) d -> p r d", p=128))

            # ---------------- Phase B: attention ----------------
            with (
                tc.tile_pool(name="pb_l", bufs=2, space="PSUM") as pb_l,
                tc.tile_pool(name="pb_t", bufs=3, space="PSUM") as pb_t,
                tc.tile_pool(name="pb_c", bufs=1, space="PSUM") as pb_c,
                tc.tile_pool(name="praw_p", bufs=4) as praw_p,
                tc.tile_pool(name="p16_p", bufs=4) as p16_p,
                tc.tile_pool(name="pT_p", bufs=2) as pT_p,
                tc.tile_pool(name="rs_p", bufs=8) as rs_p,
            ):
                for h in range(HL):
                    pr = h // 2
                    p0 = (h % 2) * 64
                    for qg in range(QB // 4):       # groups of 4 q-stripes
                        pT16 = pT_p.tile([128, KC, 512], F16, tag="pT16")
                        for qi in range(4):
                            qb = qg * 4 + qi
                            praw = praw_p.tile([128, KB, 512], F16, tag="praw")
                            for half in range(2):
                                psl = pb_l.tile([128, 2, 512], F32, tag="psl")
                                for kk in range(2):
                                    kb = half * 2 + kk
                                    nc.tensor.matmul(
                                        psl[:, kk, :],
                                        qT16[p0:p0 + 64, pr, qb * 128:(qb + 1) * 128],
                                        kT16[p0:p0 + 64, pr, kb * 512:(kb + 1) * 512],
                                        start=True,
                                        stop=True,
                                    )
                                nc.scalar.activation(
                                    praw[:, half * 2:(half + 1) * 2, :],
                                    psl,
                                    mybir.ActivationFunctionType.Exp,
                                )
                            p16 = p16_p.tile([128, S], F16, tag="p16")
                            rs = rs_p.tile([128, 1], F32, tag="rs")
                            nc.vector.scalar_tensor_tensor(
                                out=p16,
                                in0=praw.rearrange("p a b -> p (a b)"),
                                scalar=1.0,
                                in1=mk[:, qb, :],
                                op0=mybir.AluOpType.mult,
                                op1=mybir.AluOpType.mult,
                                accum_out=rs,
                            )
                            rcp = rs_p.tile([128, 1], F32, tag="rcp")
                            nc.vector.reciprocal(rcp, rs)
                            nc.vector.tensor_scalar_mul(p16, p16, rcp)
                            nc.sync.dma_start(
                                attn[h, qb * 128:(qb + 1) * 128, :], p16
                            )
                            for kb in range(KB):
                                pst = pb_t.tile([128, 4, 128], F16, tag="pst")
                                for j in range(4):
                                    nc.tensor.transpose(
                                        pst[:, j, :],
                                        p16[:, (kb * 4 + j) * 128:(kb * 4 + j + 1) * 128],
                                        ident,
                                    )
                                nc.any.tensor_copy(
                                    pT16[:, kb * 4:(kb + 1) * 4, qi * 128:(qi + 1) * 128],
                                    pst,
                                )
                        psc = pb_c.tile([64, 512], F32, tag="psc")
                        for kc in range(KC):
                            nc.tensor.matmul(
                                psc,
                                v16[:, kc, h * 64:(h + 1) * 64],
                                pT16[:, kc, :],
                                start=(kc == 0),
                                stop=(kc == KC - 1),
                            )
                        nc.scalar.copy(
                            ctxT16[p0:p0 + 64, pr, qg * 512:(qg + 1) * 512], psc
                        )

            # ---------------- Phase C: output projection ----------------
            with (
                tc.tile_pool(name="pc", bufs=4, space="PSUM") as pc,
                tc.tile_pool(name="osb_p", bufs=3) as osb_p,
            ):
                for qt in range(QB):
                    for n in range(2):
                        ps = pc.tile([128, 512], F32, tag="pc")
                        for p4 in range(PAIRS):
                            nc.tensor.matmul(
                                ps,
                                ctxT16[:, p4, qt * 128:(qt + 1) * 128],
                                wo16[:, p4, n * 512:(n + 1) * 512],
                                start=(p4 == 0),
                                stop=(p4 == PAIRS - 1),
                            )
                        osb = osb_p.tile([128, 512], F32, tag="osb")
                        nc.any.tensor_copy(osb, ps)
                        nc.sync.dma_start(
                            outp[qt * 128:(qt + 1) * 128, n * 512:(n + 1) * 512],
                            osb,
                        )

    nc.compile()
    return nc


def _get_nc():
    global _compiled_nc
    if _compiled_nc is None:
        _compiled_nc = _build_nc()
    return _compiled_nc


def _prep_in_maps(queries, keys, values, mask, Wq, bq, Wk, bk, Wv, bv, Wo, bo):
    """Host-side sharding / layout prep. Core c = (batch c//2, head-half c%2)."""
    scale = 1.0 / np.sqrt(np.float32(DH))
    maskinv_all = (1.0 - mask[:, 0]).astype(np.float16)  # (B,S,S) 1=keep
    xq = [np.ascontiguousarray(queries[b].T).astype(np.float16) for b in range(B)]
    xk = [np.ascontiguousarray(keys[b].T).astype(np.float16) for b in range(B)]
    xv = [np.ascontiguousarray(values[b].T).astype(np.float16) for b in range(B)]
    in_maps = []
    for c in range(N_CORES):
        b, g = c // 2, c % 2
        cols = slice(g * DL, (g + 1) * DL)
        in_maps.append({
            "xqt": xq[b],
            "xkt": xk[b],
            "xvt": xv[b],
            "wq": (Wq[:, cols] * scale).astype(np.float16),
            "wk": Wk[:, cols].astype(np.float16),
            "wv": Wv[:, cols].astype(np.float16),
            "bq": np.ascontiguousarray(bq[cols] * scale),
            "bk": np.ascontiguousarray(bk[cols]),
            "bv": np.ascontiguousarray(bv[cols])[None, :],
            "wo": np.ascontiguousarray(Wo[g * DL:(g + 1) * DL, :]).astype(np.float16),
            "maskinv": maskinv_all[b],
        })
    return in_maps


def _run(inputs, trace=False, trace_kwargs=None):
    inputs = {k: np.asarray(v, dtype=np.float32) for k, v in inputs.items()}
    nc = _get_nc()
    in_maps = _prep_in_maps(**inputs)
    kw = {}
    if trace:
        kw = dict(trace=True, trace_kwargs=trace_kwargs or {})
    res = run_bass_kernel_spmd(nc, in_maps, list(range(N_CORES)), **kw)

    attn_w = np.empty((B, H, S, S), np.float32)
    out = np.empty((B, S, D), np.float32)
    bo = inputs["bo"]
    for c in range(N_CORES):
        b, g = c // 2, c % 2
        attn_w[b, g * HL:(g + 1) * HL] = res.results[c]["attn"].astype(np.float32)
    for b in range(B):
        out[b] = res.results[2 * b]["outp"] + res.results[2 * b + 1]["outp"] + bo
    return (out, attn_w), res


def kernel(**inputs):
    (out, attn_w), _ = _run(inputs, trace=False)
    return out, attn_w
